# revision 2
# baseline (speedup 1.0000x reference)
"""Multi-head attention (B=2, S=2048, D=2048, H=16, Dh=128) on 8 TRN2 NeuronCores.

Tensor-parallel over heads: core c owns heads {2c, 2c+1}.

Per-core pipeline (all matmuls float32r):
  Phase A: QKV projection from replicated x^T.
           Q^T, K^T produced in [head_dim, token] layout (softmax scale folded
           into w_q on host); V produced natural [token, head_dim].
  Phase B: attention per (local head, batch, 512-wide q tile), transposed
           formulation: S^T[k,q] tiles via K^T-stationary matmuls; exp on
           ScalarE straight out of PSUM (no max subtraction -- logits are
           N(0,1)-scaled); denominator via ones-vector matmul over the
           DVE-accumulated sum of P^T tiles; PV^T accumulation with V chunks
           stationary giving combined^T [head_dim, token]; division by the
           denominator via K=1 outer-product broadcast + DVE multiply.
  A2A:     one AllToAll per local head moves combined^T from head-sharded to
           token-sharded (core c ends up with all 2048 combined dims for its
           512 tokens). w_out^T rows are pre-permuted on host to match the
           (even heads | odd heads) order the two collectives produce.
  Phase C: out-projection for the core's 512 tokens, streaming w_out^T.

Host: shards/transposes weights, replicates x^T, concatenates per-core token
slices into the full (2, 2048, 2048) output.
"""

import sys

import numpy as np

for _p in ("/opt/trn_rl_repo", "/root/.axon_site/_ro/trn_rl_repo"):
    if _p not in sys.path:
        sys.path.insert(0, _p)

from concourse import bacc, bass, mybir, tile
from concourse.bass_utils import run_bass_kernel_spmd

B = 2
S = 2048
D = 2048
H = 16
DH = 128
NC = 8
HL = 2  # heads per core
T = B * S  # 4096 tokens
TPC = T // NC  # 512 tokens per core

F32 = mybir.dt.float32
F32R = mybir.dt.float32r
EXP = mybir.ActivationFunctionType.Exp

_graph_cache = {}


def build_graph(mm_dt=F32R):
    nc = bacc.Bacc(
        "TRN2",
        target_bir_lowering=False,
        debug=False,
        enable_asserts=False,
        num_devices=NC,
    )
    xT = nc.dram_tensor("xT", [D, T], F32, kind="ExternalInput")
    wqkvT = nc.dram_tensor("wqkvT", [D, 3 * HL * DH], F32, kind="ExternalInput")
    woutT = nc.dram_tensor("woutT", [D, D], F32, kind="ExternalInput")
    out_ext = nc.dram_tensor("out", [TPC, D], F32, kind="ExternalOutput")

    DC = D // 128  # 16 contraction chunks of 128

    with tile.TileContext(nc) as tc:
        with (
            tc.tile_pool(name="constp", bufs=1) as constp,
            tc.tile_pool(name="dramp", bufs=1, space="DRAM") as dramp,
        ):
            ones_col = constp.tile([128, 1], F32)
            ones_row = constp.tile([1, 128], F32)
            nc.vector.memset(ones_col[:], 1.0)
            nc.vector.memset(ones_row[:], 1.0)

            a2a_send = [
                dramp.tile([NC, 128, TPC], F32, name=f"a2a_send{h}") for h in range(HL)
            ]
            a2a_recv = [
                dramp.tile([NC, 128, TPC], F32, name=f"a2a_recv{h}") for h in range(HL)
            ]

            with tc.tile_pool(name="qkvp", bufs=1) as qkvp:
                # persistent activations for phase B
                QT = qkvp.tile([128, HL, T], mm_dt)  # [d, hl, tok]
                KT = qkvp.tile([128, HL, T], mm_dt)
                V = qkvp.tile([128, T // 128, HL * DH], mm_dt)  # [tok%128, chunk, f]

                # ---------------- Phase A: QKV projection ----------------
                with (
                    tc.tile_pool(name="scrA", bufs=1) as scrA,
                    tc.tile_pool(name="xtp", bufs=5) as xtp,
                    tc.tile_pool(name="psA", bufs=2, space="PSUM") as psA,
                ):
                    wqkv_s = scrA.tile([128, DC, 3 * HL * DH], mm_dt)
                    nc.sync.dma_start(
                        out=wqkv_s[:],
                        in_=wqkvT.ap()
                        .bitcast(mm_dt)
                        .rearrange("(dc p) f -> p dc f", p=128),
                    )
                    for t in range(T // 512):
                        # x^T token slice in 4 quarter tiles of 4 chunks each
                        xq = []
                        for qh in range(4):
                            xq_t = xtp.tile([128, 4, 512], mm_dt, tag="xq", name="xq")
                            nc.sync.dma_start(
                                out=xq_t[:],
                                in_=xT.ap()[
                                    qh * 512 : (qh + 1) * 512,
                                    t * 512 : (t + 1) * 512,
                                ]
                                .bitcast(mm_dt)
                                .rearrange("(dc p) f -> p dc f", p=128),
                            )
                            xq.append(xq_t)

                        # Q^T / K^T: psum[f=128, tok=512]
                        for ft in range(2 * HL):  # q0 q1 k0 k1
                            ps = psA.tile([128, 512], F32, tag="psqk")
                            for dc in range(DC):
                                nc.tensor.matmul(
                                    ps[:],
                                    wqkv_s[:, dc, ft * 128 : (ft + 1) * 128],
                                    xq[dc // 4][:, dc % 4, :],
                                    start=(dc == 0),
                                    stop=(dc == DC - 1),
                                )
                            dest = QT if ft < HL else KT
                            hl = ft % HL
                            nc.scalar.copy(dest[:, hl, t * 512 : (t + 1) * 512], ps[:])
                        # V natural: psum[tok=128, f=256]
                        for sub in range(4):
                            psv = psA.tile([128, HL * DH], F32, tag="psv")
                            for dc in range(DC):
                                nc.tensor.matmul(
                                    psv[:],
                                    xq[dc // 4][:, dc % 4, sub * 128 : (sub + 1) * 128],
                                    wqkv_s[:, dc, 2 * HL * DH : 3 * HL * DH],
                                    start=(dc == 0),
                                    stop=(dc == DC - 1),
                                )
                            nc.scalar.copy(V[:, t * 4 + sub, :], psv[:])

                # -------- Phases B (attention + A2A) and C (out proj) --------
                with tc.tile_pool(name="woutp", bufs=3) as woutp:
                    # stream w_out^T in 8 half-group tiles; the first ~3
                    # prefetch during attention, the rest flow as slots free.
                    whalf = []
                    for g in range(4):
                        for hf in range(2):
                            wtile = woutp.tile(
                                [128, 8, 512], mm_dt, tag="wout", name="wout"
                            )
                            nc.sync.dma_start(
                                out=wtile[:],
                                in_=woutT.ap()[
                                    hf * 1024 : (hf + 1) * 1024,
                                    g * 512 : (g + 1) * 512,
                                ]
                                .bitcast(mm_dt)
                                .rearrange("(dc p) f -> p dc f", p=128),
                            )
                            whalf.append(wtile)

                    with (
                        tc.tile_pool(name="pB", bufs=2) as pB,
                        tc.tile_pool(name="psB", bufs=2, space="PSUM") as psB,
                    ):
                        n_k = S // 128  # 16 k tiles per (b, head)
                        for hl in range(HL):
                            combT = pB.tile(
                                [128, T], F32, tag="combT", name="combT", bufs=1
                            )
                            for b in range(B):
                                for qt in range(S // 512):
                                    q_sl = slice(
                                        b * S + qt * 512, b * S + (qt + 1) * 512
                                    )
                                    ps_o = psB.tile([128, 512], F32, tag="ps_o")
                                    l_acc = pB.tile([128, 512], F32, tag="lacc")
                                    for kt in range(n_k):
                                        ps_s = psB.tile([128, 512], F32, tag="ps_s")
                                        nc.tensor.matmul(
                                            ps_s[:],
                                            KT[
                                                :,
                                                hl,
                                                b * S + kt * 128 : b * S
                                                + (kt + 1) * 128,
                                            ],
                                            QT[:, hl, q_sl],
                                            start=True,
                                            stop=True,
                                        )
                                        pt = pB.tile([128, 512], mm_dt, tag="pt")
                                        nc.scalar.activation(pt[:], ps_s[:], EXP)
                                        if kt == 0:
                                            nc.vector.tensor_copy(
                                                l_acc[:], pt[:].bitcast(F32)
                                            )
                                        else:
                                            nc.vector.tensor_add(
                                                l_acc[:], l_acc[:], pt[:].bitcast(F32)
                                            )
                                        nc.tensor.matmul(
                                            ps_o[:],
                                            V[
                                                :,
                                                b * (S // 128) + kt,
                                                hl * DH : (hl + 1) * DH,
                                            ],
                                            pt[:],
                                            start=(kt == 0),
                                            stop=(kt == n_k - 1),
                                        )
                                    ps_l = psB.tile([1, 512], F32, tag="ps_l")
                                    nc.tensor.matmul(
                                        ps_l[:],
                                        ones_col[:],
                                        l_acc[:],
                                        start=True,
                                        stop=True,
                                    )
                                    rl = pB.tile([1, 512], F32, tag="rl")
                                    nc.vector.reciprocal(rl[:], ps_l[:])
                                    ps_b = psB.tile([128, 512], F32, tag="ps_b")
                                    nc.tensor.matmul(
                                        ps_b[:],
                                        ones_row[:],
                                        rl[:],
                                        start=True,
                                        stop=True,
                                    )
                                    rlb = pB.tile([128, 512], F32, tag="rlb")
                                    nc.vector.tensor_copy(rlb[:], ps_b[:])
                                    nc.vector.tensor_mul(
                                        combT[:, q_sl], ps_o[:], rlb[:]
                                    )
                            # ship this head's combined^T (shard j = core j's
                            # tokens), then redistribute head->token sharding.
                            nc.sync.dma_start(
                                out=a2a_send[hl].rearrange("j p f -> p j f"),
                                in_=combT[:, :].rearrange("p (j f) -> p j f", j=NC),
                            )
                            nc.gpsimd.collective_compute(
                                "AllToAll",
                                mybir.AluOpType.bypass,
                                replica_groups=[list(range(NC))],
                                ins=[a2a_send[hl][:]],
                                outs=[a2a_recv[hl][:]],
                            )

                    # ---------------- Phase C: out projection ----------------
                    with (
                        tc.tile_pool(name="pC", bufs=1) as pC,
                        tc.tile_pool(name="evC", bufs=2) as evC,
                        tc.tile_pool(name="psC", bufs=2, space="PSUM") as psC,
                    ):
                        comb_in = []
                        for cc in range(DC):
                            hi, blk = (0, cc) if cc < 8 else (1, cc - 8)
                            ctile = pC.tile(
                                [128, TPC],
                                mm_dt,
                                tag="comb_in",
                                name="comb_in",
                                bufs=DC,
                            )
                            nc.sync.dma_start(
                                out=ctile[:], in_=a2a_recv[hi][blk].bitcast(mm_dt)
                            )
                            comb_in.append(ctile)
                        for g in range(4):
                            for ts in range(TPC // 128):
                                ps = psC.tile([128, 512], F32, tag="psc")
                                for cc in range(DC):
                                    nc.tensor.matmul(
                                        ps[:],
                                        comb_in[cc][:, ts * 128 : (ts + 1) * 128],
                                        whalf[g * 2 + cc // 8][:, cc % 8, :],
                                        start=(cc == 0),
                                        stop=(cc == DC - 1),
                                    )
                                ev = evC.tile([128, 512], F32, tag="ev")
                                nc.scalar.copy(ev[:], ps[:])
                                nc.sync.dma_start(
                                    out=out_ext.ap()[
                                        ts * 128 : (ts + 1) * 128,
                                        g * 512 : (g + 1) * 512,
                                    ],
                                    in_=ev[:],
                                )
    nc.finalize()
    return nc


def prep_inputs(x, w_qkv, w_out):
    """Host-side sharding. Returns list of per-core input dicts."""
    x = np.asarray(x, dtype=np.float32)
    w_qkv = np.asarray(w_qkv, dtype=np.float32)
    w_out = np.asarray(w_out, dtype=np.float32)

    xT = np.ascontiguousarray(x.reshape(T, D).T)  # [D, T]

    # w_out^T with rows permuted to (even heads | odd heads)
    woutT = w_out.T  # [cin, dout], cin = h*DH + d
    perm = [2 * i for i in range(8)] + [2 * i + 1 for i in range(8)]
    woutT_perm = np.ascontiguousarray(
        np.concatenate([woutT[h * DH : (h + 1) * DH] for h in perm], axis=0)
    )

    scale = np.float32(1.0 / np.sqrt(DH))
    in_maps = []
    for c in range(NC):
        h0 = HL * c
        wq = w_qkv[h0 * DH : (h0 + HL) * DH] * scale  # [256, D]
        wk = w_qkv[H * DH + h0 * DH : H * DH + (h0 + HL) * DH]
        wv = w_qkv[2 * H * DH + h0 * DH : 2 * H * DH + (h0 + HL) * DH]
        wqkvT = np.ascontiguousarray(np.concatenate([wq, wk, wv], axis=0).T)  # [D,768]
        in_maps.append({"xT": xT, "wqkvT": wqkvT, "woutT": woutT_perm})
    return in_maps


def run(x, w_qkv, w_out, mm_dt=F32R, trace=False, tmpdir=None):
    key = str(mm_dt)
    if key not in _graph_cache:
        _graph_cache[key] = build_graph(mm_dt)
    nc = _graph_cache[key]
    in_maps = prep_inputs(x, w_qkv, w_out)
    res = run_bass_kernel_spmd(
        nc, in_maps, core_ids=list(range(NC)), trace=trace, tmpdir=tmpdir
    )
    out = np.concatenate([res.results[c]["out"] for c in range(NC)], axis=0)
    return out.reshape(B, S, D).astype(np.float32), res


def kernel(x, w_qkv, w_out):
    out, _ = run(x, w_qkv, w_out)
    return out
